# revision 18
# baseline (speedup 1.0000x reference)
"""Trainium2 Bass kernel for nn_BilinearFusion.

out[b] = sum_h [ x1_h(b)·W1_h + b1_h + x2_h(b)·W2_h + x2_h(b)^T W3_h x1_h(b) ]

Host-side staging: shard batch across 8 cores; cast x1/x2 to bf16 and lay
them out pre-transposed per head (xt[i, h, b] = x[b, h*128+i]) in batch-major
1MB chunks so the device only does contiguous DMA loads.

Device (per core, 2048 rows = 4 batches x 512), balanced across engines so
DMA-in (~24us at ~340 GB/s) is the only near-critical resource:
  per batch, per head h:
    PE : Yt_h = W3_h^T @ x1t_h            (512-cycle stream, fp32 PSUM)
    PE : rps[1,b] += W1_h^T @ x1t_h       (t1, M=1 accumulate)
    consume Yt (folds t2 = W2_h·x2_h via per-partition scalar):
      heads in PATH_A: DVE scalar_tensor_tensor
                       prod = (Yt + W2col) * x2t      (PSUM src, 1x)
      else:            ACT  s = Identity(Yt + W2col)  -> bf16 SBUF
                       DVE  prod = s * x2t            (bf16 SBUF, 2x)
    PE : rps[1,b] += ones^T @ prod        (reduce over o; lagged by RED_LAG
                                           heads so PE never waits on DVE)
  DVE copies rps -> res slice; one sync DMA stores res.  Host adds sum(b1).
"""

import numpy as np
import ml_dtypes

import concourse.bass as bass
import concourse.tile as tile
from concourse import bacc, mybir
from concourse.bass_utils import run_bass_kernel_spmd

BF16 = ml_dtypes.bfloat16

B, D, HEAD, DIM = 16384, 1024, 8, 128
NCORES = 8
ROWS = B // NCORES          # 2048 rows per core
P = 128
BATCH = 512                 # rows per batch (moving free dim of matmuls)
NB = ROWS // BATCH          # 4 batches

PATH_A = (7,)               # heads consumed by DVE straight from PSUM
T1_DVE = ()                 # heads whose t1 term folds into prods on the DVE
RED_LAG = 2                 # reduce matmuls trail the pair-sums by this many
N_WARM = 5                  # PE clock-gate warm-up matmuls

_nc_cache = []


def build_nc():
    nc = bacc.Bacc(target_bir_lowering=False)
    f32 = mybir.dt.float32
    bf16 = mybir.dt.bfloat16
    Alu = mybir.AluOpType
    Act = mybir.ActivationFunctionType

    x1t_d = nc.dram_tensor("x1t", [NB, P, HEAD, BATCH], bf16,
                           kind="ExternalInput")
    x2t_d = nc.dram_tensor("x2t", [NB, P, HEAD, BATCH], bf16,
                           kind="ExternalInput")
    w3t_d = nc.dram_tensor("w3t", [DIM, HEAD, DIM], bf16, kind="ExternalInput")
    w12b_d = nc.dram_tensor("w12b", [DIM, 2, HEAD], bf16, kind="ExternalInput")
    w12f_d = nc.dram_tensor("w12f", [DIM, 2, HEAD], f32, kind="ExternalInput")
    out_d = nc.dram_tensor("out", [NB * BATCH], f32, kind="ExternalOutput")

    with tile.TileContext(nc) as tc:
        with (
            tc.tile_pool(name="const", bufs=1) as const_pool,
            tc.tile_pool(name="xt", bufs=NB) as xt_pool,
            tc.tile_pool(name="s", bufs=8) as s_pool,
            tc.tile_pool(name="prod", bufs=3) as prod_pool,
            tc.tile_pool(name="res", bufs=1) as res_pool,
            tc.tile_pool(name="yps", bufs=4, space="PSUM") as yps_pool,
            tc.tile_pool(name="rps", bufs=2, space="PSUM") as rps_pool,
        ):
            # ---- constants + all bulk loads dispatched up front ----
            # Queue discipline: a dma_start dispatch can block on DMA
            # flow-control sems, stalling everything behind it on that
            # engine's queue.  So: scalar(ACT) gets only the small weight
            # loads (done early, then pure compute); sync streams x1t
            # (HWDGE); gpsimd streams x2t (SWDGE ring, no compute there).
            ones = const_pool.tile([DIM, 1], bf16)
            nc.vector.memset(ones, 1.0)

            # weights ride the FRONT of the sync ring: a separate scalar-ring
            # load gets starved by the sync bulk stream (measured 10us for
            # 275KB), stalling the first W3 matmuls and re-throttling the PE.
            w12b = const_pool.tile([DIM, 2, HEAD], bf16)
            w12f = const_pool.tile([DIM, 2, HEAD], f32)
            w3l = const_pool.tile([DIM, HEAD, DIM], bf16)
            nc.sync.dma_start(out=w12b, in_=w12b_d[:])
            nc.sync.dma_start(out=w12f, in_=w12f_d[:])
            nc.sync.dma_start(out=w3l, in_=w3t_d[:])

            # batch 0 arrives in growing head-chunks so compute starts early;
            # batch 3 in halves to shorten the tail; middle ones as 1MB slabs.
            chunks = {0: ((0, 1), (1, 2), (2, 4), (4, 8)),
                      NB - 1: ((0, 4), (4, 8))}
            x1ts, x2ts = [], []
            for bat in range(NB):
                x1t = xt_pool.tile([P, HEAD, BATCH], bf16, tag="x1t")
                x2t = xt_pool.tile([P, HEAD, BATCH], bf16, tag="x2t")
                x1ts.append(x1t)
                x2ts.append(x2t)
                for a, b in chunks.get(bat, ((0, 8),)):
                    nc.sync.dma_start(out=x1t[:, a:b, :],
                                      in_=x1t_d[bat, :, a:b, :])
                    # late back-halves of x2 ride the idle SWDGE ring,
                    # shrinking the sync ring's stream by ~1.5MB
                    sw_a = max(a, 4) if bat >= 1 else b
                    if sw_a < b:
                        nc.gpsimd.dma_start(out=x2t[:, sw_a:b, :],
                                            in_=x2t_d[bat, :, sw_a:b, :])
                    if a < min(b, sw_a):
                        e = min(b, sw_a)
                        nc.sync.dma_start(out=x2t[:, a:e, :],
                                          in_=x2t_d[bat, :, a:e, :])

            # warm the ACT function table while DMA streams
            warm = const_pool.tile([1, 1], f32)
            nc.scalar.activation(warm, ones[0:1, 0:1], Act.Identity,
                                 bias=0.0, scale=1.0)

            # warm the PE HAM clock-gate during the DMA fill: the gate keeps
            # the array at half clock until ~4us of sustained activity, so
            # burn that time on scratch matmuls instead of the real ones.
            scratch = const_pool.tile([DIM, BATCH], bf16)
            nc.vector.memset(scratch, 0.0)
            wps = yps_pool.tile([DIM, BATCH], f32, tag="warm", bufs=1)
            for _ in range(N_WARM):
                nc.tensor.matmul(wps, scratch[:, 0:DIM], scratch,
                                 start=True, stop=True)

            res = res_pool.tile([1, NB * BATCH], f32)

            # deferred reduce matmuls + result copies, so PE never waits on
            # the DVE round-trip.  entry: (fn,) thunk emitting one reduce.
            pending = []

            def drain(n):
                while len(pending) > n:
                    pending.pop(0)()

            for bat in range(NB):
                x1t, x2t = x1ts[bat], x2ts[bat]
                rps = rps_pool.tile([1, BATCH], f32)
                prods = prod_pool.tile([DIM, HEAD, BATCH], bf16, tag="p")
                pairs = prod_pool.tile([DIM, HEAD // 2, BATCH], bf16,
                                       tag="q")
                for h in range(HEAD):
                    yps = yps_pool.tile([DIM, BATCH], f32)
                    nc.tensor.matmul(yps, w3l[:, h, :], x1t[:, h, :],
                                     start=True, stop=True)
                    if h not in T1_DVE:
                        nc.tensor.matmul(rps, w12b[:, 0, h:h + 1],
                                         x1t[:, h, :],
                                         start=(h == 0), stop=False)
                    prod = prods[:, h, :]
                    if h in PATH_A:
                        nc.vector.scalar_tensor_tensor(
                            prod, yps, w12f[:, 1, h:h + 1], x2t[:, h, :],
                            op0=Alu.add, op1=Alu.mult)
                    else:
                        s = s_pool.tile([DIM, BATCH], bf16, tag="s")
                        nc.scalar.activation(s, yps, Act.Identity,
                                             bias=w12f[:, 1, h:h + 1],
                                             scale=1.0)
                        nc.vector.tensor_mul(prod, s, x2t[:, h, :])
                    if h in T1_DVE:
                        # fold t1 into the prod slab in place on the DVE
                        nc.vector.scalar_tensor_tensor(
                            prod, x1t[:, h, :], w12f[:, 0, h:h + 1], prod,
                            op0=Alu.mult, op1=Alu.add)
                    if h % 2 == 1:
                        # halve the PE's reduce streams: sum slab pairs on
                        # the DVE (bf16 SBUF, 2x mode), reduce 4 slabs only
                        pair = pairs[:, h // 2, :]
                        nc.vector.tensor_add(pair, prods[:, h - 1, :],
                                             prod)
                        def red(rps=rps, pair=pair, h=h, bat=bat):
                            nc.tensor.matmul(rps, ones, pair, start=False,
                                             stop=(h == HEAD - 1))
                            if h == HEAD - 1:
                                nc.scalar.copy(
                                    res[:, bat * BATCH:(bat + 1) * BATCH],
                                    rps)
                        pending.append(red)
                        drain(RED_LAG)
            drain(0)

            nc.sync.dma_start(out=out_d[:], in_=res)

    nc.finalize()
    return nc


def _prep_weights(W1, W2, W3):
    # W3 is [h, o, i]; lhsT needs [i (partitions), h, o]
    w3t = np.ascontiguousarray(
        np.transpose(np.asarray(W3), (2, 0, 1))).astype(BF16)
    w12f = np.empty((DIM, 2, HEAD), dtype=np.float32)
    w12f[:, 0, :] = np.asarray(W1).T   # [i, h]
    w12f[:, 1, :] = np.asarray(W2).T   # [o, h]
    return w3t, w12f.astype(BF16), w12f


def _prep_x(x):
    """[B, D] fp32 -> per-core [NB, P, HEAD, BATCH] bf16, pre-transposed."""
    xb = np.asarray(x, dtype=np.float32).astype(BF16)
    # [core, bat, b, h, i] -> [core, bat, i, h, b]
    v = xb.reshape(NCORES, NB, BATCH, HEAD, DIM).transpose(0, 1, 4, 3, 2)
    return np.ascontiguousarray(v)


def _in_maps(x1, x2, W1, W2, W3):
    w3t, w12b, w12f = _prep_weights(W1, W2, W3)
    x1t = _prep_x(x1)
    x2t = _prep_x(x2)
    return [
        {"x1t": x1t[c], "x2t": x2t[c], "w3t": w3t,
         "w12b": w12b, "w12f": w12f}
        for c in range(NCORES)
    ]


def kernel(x1, x2, W1, b1, W2, W3):
    if not _nc_cache:
        _nc_cache.append(build_nc())
    nc = _nc_cache[0]

    c_b1 = float(np.asarray(b1, dtype=np.float64).sum())
    in_maps = _in_maps(x1, x2, W1, W2, W3)

    res = run_bass_kernel_spmd(nc, in_maps, core_ids=list(range(NCORES)))
    out = np.concatenate(
        [res.results[c]["out"].reshape(-1) for c in range(NCORES)])
    return (out + np.float32(c_b1)).astype(np.float32)


# revision 19
# speedup vs baseline: 1.0443x; 1.0443x over previous
"""Trainium2 Bass kernel for nn_BilinearFusion.

out[b] = sum_h [ x1_h(b)·W1_h + b1_h + x2_h(b)·W2_h + x2_h(b)^T W3_h x1_h(b) ]

Host-side staging: shard batch across 8 cores; cast x1/x2 to bf16 and lay
them out pre-transposed per head (xt[i, h, b] = x[b, h*128+i]) in batch-major
1MB chunks so the device only does contiguous DMA loads.

Device (per core, 2048 rows = 4 batches x 512), balanced across engines so
DMA-in (~24us at ~340 GB/s) is the only near-critical resource:
  per batch, per head h:
    PE : Yt_h = W3_h^T @ x1t_h            (512-cycle stream, fp32 PSUM)
    PE : rps[1,b] += W1_h^T @ x1t_h       (t1, M=1 accumulate)
    consume Yt (folds t2 = W2_h·x2_h via per-partition scalar):
      heads in PATH_A: DVE scalar_tensor_tensor
                       prod = (Yt + W2col) * x2t      (PSUM src, 1x)
      else:            ACT  s = Identity(Yt + W2col)  -> bf16 SBUF
                       DVE  prod = s * x2t            (bf16 SBUF, 2x)
    PE : rps[1,b] += ones^T @ prod        (reduce over o; lagged by RED_LAG
                                           heads so PE never waits on DVE)
  DVE copies rps -> res slice; one sync DMA stores res.  Host adds sum(b1).
"""

import numpy as np
import ml_dtypes

import concourse.bass as bass
import concourse.tile as tile
from concourse import bacc, mybir
from concourse.bass_utils import run_bass_kernel_spmd

BF16 = ml_dtypes.bfloat16

B, D, HEAD, DIM = 16384, 1024, 8, 128
NCORES = 8
ROWS = B // NCORES          # 2048 rows per core
P = 128
BATCH = 512                 # rows per batch (moving free dim of matmuls)
NB = ROWS // BATCH          # 4 batches

PATH_A = (7,)               # heads consumed by DVE straight from PSUM
T1_DVE = ()                 # heads whose t1 term folds into prods on the DVE
RED_LAG = 2                 # reduce matmuls trail the pair-sums by this many
N_WARM = 10                 # PE clock-gate warm-up matmuls

_nc_cache = []


def build_nc():
    nc = bacc.Bacc(target_bir_lowering=False)
    f32 = mybir.dt.float32
    bf16 = mybir.dt.bfloat16
    Alu = mybir.AluOpType
    Act = mybir.ActivationFunctionType

    x1t_d = nc.dram_tensor("x1t", [NB, P, HEAD, BATCH], bf16,
                           kind="ExternalInput")
    x2t_d = nc.dram_tensor("x2t", [NB, P, HEAD, BATCH], bf16,
                           kind="ExternalInput")
    w3t_d = nc.dram_tensor("w3t", [DIM, HEAD, DIM], bf16, kind="ExternalInput")
    w12b_d = nc.dram_tensor("w12b", [DIM, 2, HEAD], bf16, kind="ExternalInput")
    w12f_d = nc.dram_tensor("w12f", [DIM, 2, HEAD], f32, kind="ExternalInput")
    out_d = nc.dram_tensor("out", [NB * BATCH], f32, kind="ExternalOutput")

    with tile.TileContext(nc) as tc:
        with (
            tc.tile_pool(name="const", bufs=1) as const_pool,
            tc.tile_pool(name="xt", bufs=NB) as xt_pool,
            tc.tile_pool(name="s", bufs=8) as s_pool,
            tc.tile_pool(name="prod", bufs=3) as prod_pool,
            tc.tile_pool(name="res", bufs=1) as res_pool,
            tc.tile_pool(name="yps", bufs=4, space="PSUM") as yps_pool,
            tc.tile_pool(name="rps", bufs=2, space="PSUM") as rps_pool,
        ):
            # ---- constants + all bulk loads dispatched up front ----
            # Queue discipline: a dma_start dispatch can block on DMA
            # flow-control sems, stalling everything behind it on that
            # engine's queue.  So: scalar(ACT) gets only the small weight
            # loads (done early, then pure compute); sync streams x1t
            # (HWDGE); gpsimd streams x2t (SWDGE ring, no compute there).
            ones = const_pool.tile([DIM, 1], bf16)
            nc.vector.memset(ones, 1.0)

            # weights ride the FRONT of the sync ring: a separate scalar-ring
            # load gets starved by the sync bulk stream (measured 10us for
            # 275KB), stalling the first W3 matmuls and re-throttling the PE.
            w12b = const_pool.tile([DIM, 2, HEAD], bf16)
            w12f = const_pool.tile([DIM, 2, HEAD], f32)
            w3l = const_pool.tile([DIM, HEAD, DIM], bf16)
            nc.sync.dma_start(out=w3l, in_=w3t_d[:])

            # batch 0 arrives in growing head-chunks so compute starts early;
            # batch 3 in halves to shorten the tail; middle ones as 1MB slabs.
            chunks = {0: ((0, 1), (1, 2), (2, 4), (4, 8)),
                      NB - 1: ((0, 4), (4, 8))}
            x1ts, x2ts = [], []
            w12_loaded = False
            for bat in range(NB):
                x1t = xt_pool.tile([P, HEAD, BATCH], bf16, tag="x1t")
                x2t = xt_pool.tile([P, HEAD, BATCH], bf16, tag="x2t")
                x1ts.append(x1t)
                x2ts.append(x2t)
                for a, b in chunks.get(bat, ((0, 8),)):
                    nc.sync.dma_start(out=x1t[:, a:b, :],
                                      in_=x1t_d[bat, :, a:b, :])
                    if not w12_loaded:
                        # small w12 loads slot in after the first chunk pair
                        # (needed only once t1_0 / the first bias-add run)
                        w12_loaded = True
                        nc.sync.dma_start(out=w12b, in_=w12b_d[:])
                        nc.sync.dma_start(out=w12f, in_=w12f_d[:])
                    # late back-halves of x2 ride the idle SWDGE ring,
                    # shrinking the sync ring's stream by ~1.5MB
                    sw_a = max(a, 4) if bat >= 1 else b
                    if sw_a < b:
                        nc.gpsimd.dma_start(out=x2t[:, sw_a:b, :],
                                            in_=x2t_d[bat, :, sw_a:b, :])
                    if a < min(b, sw_a):
                        e = min(b, sw_a)
                        nc.sync.dma_start(out=x2t[:, a:e, :],
                                          in_=x2t_d[bat, :, a:e, :])

            # warm the ACT function table while DMA streams
            warm = const_pool.tile([1, 1], f32)
            nc.scalar.activation(warm, ones[0:1, 0:1], Act.Identity,
                                 bias=0.0, scale=1.0)

            # warm the PE HAM clock-gate during the DMA fill: the gate keeps
            # the array at half clock until ~4us of sustained activity, so
            # burn that time on scratch matmuls instead of the real ones.
            scratch = const_pool.tile([DIM, BATCH], bf16)
            nc.vector.memset(scratch, 0.0)
            wps = yps_pool.tile([DIM, BATCH], f32, tag="warm", bufs=1)
            for _ in range(N_WARM):
                nc.tensor.matmul(wps, scratch[:, 0:DIM], scratch,
                                 start=True, stop=True)

            res = res_pool.tile([1, NB * BATCH], f32)

            # deferred reduce matmuls + result copies, so PE never waits on
            # the DVE round-trip.  entry: (fn,) thunk emitting one reduce.
            pending = []

            def drain(n):
                while len(pending) > n:
                    pending.pop(0)()

            for bat in range(NB):
                x1t, x2t = x1ts[bat], x2ts[bat]
                rps = rps_pool.tile([1, BATCH], f32)
                prods = prod_pool.tile([DIM, HEAD, BATCH], bf16, tag="p")
                pairs = prod_pool.tile([DIM, HEAD // 2, BATCH], bf16,
                                       tag="q")
                for h in range(HEAD):
                    yps = yps_pool.tile([DIM, BATCH], f32)
                    nc.tensor.matmul(yps, w3l[:, h, :], x1t[:, h, :],
                                     start=True, stop=True)
                    if h not in T1_DVE:
                        nc.tensor.matmul(rps, w12b[:, 0, h:h + 1],
                                         x1t[:, h, :],
                                         start=(h == 0), stop=False)
                    prod = prods[:, h, :]
                    if h in PATH_A:
                        nc.vector.scalar_tensor_tensor(
                            prod, yps, w12f[:, 1, h:h + 1], x2t[:, h, :],
                            op0=Alu.add, op1=Alu.mult)
                    else:
                        s = s_pool.tile([DIM, BATCH], bf16, tag="s")
                        nc.scalar.activation(s, yps, Act.Identity,
                                             bias=w12f[:, 1, h:h + 1],
                                             scale=1.0)
                        nc.vector.tensor_mul(prod, s, x2t[:, h, :])
                    if h in T1_DVE:
                        # fold t1 into the prod slab in place on the DVE
                        nc.vector.scalar_tensor_tensor(
                            prod, x1t[:, h, :], w12f[:, 0, h:h + 1], prod,
                            op0=Alu.mult, op1=Alu.add)
                    if h % 2 == 1:
                        # halve the PE's reduce streams: sum slab pairs on
                        # the DVE (bf16 SBUF, 2x mode), reduce 4 slabs only
                        pair = pairs[:, h // 2, :]
                        nc.vector.tensor_add(pair, prods[:, h - 1, :],
                                             prod)
                        def red(rps=rps, pair=pair, h=h, bat=bat):
                            nc.tensor.matmul(rps, ones, pair, start=False,
                                             stop=(h == HEAD - 1))
                            if h == HEAD - 1:
                                nc.scalar.copy(
                                    res[:, bat * BATCH:(bat + 1) * BATCH],
                                    rps)
                        pending.append(red)
                        drain(RED_LAG)
            drain(0)

            nc.sync.dma_start(out=out_d[:], in_=res)

    nc.finalize()
    return nc


def _prep_weights(W1, W2, W3):
    # W3 is [h, o, i]; lhsT needs [i (partitions), h, o]
    w3t = np.ascontiguousarray(
        np.transpose(np.asarray(W3), (2, 0, 1))).astype(BF16)
    w12f = np.empty((DIM, 2, HEAD), dtype=np.float32)
    w12f[:, 0, :] = np.asarray(W1).T   # [i, h]
    w12f[:, 1, :] = np.asarray(W2).T   # [o, h]
    return w3t, w12f.astype(BF16), w12f


def _prep_x(x):
    """[B, D] fp32 -> per-core [NB, P, HEAD, BATCH] bf16, pre-transposed."""
    xb = np.asarray(x, dtype=np.float32).astype(BF16)
    # [core, bat, b, h, i] -> [core, bat, i, h, b]
    v = xb.reshape(NCORES, NB, BATCH, HEAD, DIM).transpose(0, 1, 4, 3, 2)
    return np.ascontiguousarray(v)


def _in_maps(x1, x2, W1, W2, W3):
    w3t, w12b, w12f = _prep_weights(W1, W2, W3)
    x1t = _prep_x(x1)
    x2t = _prep_x(x2)
    return [
        {"x1t": x1t[c], "x2t": x2t[c], "w3t": w3t,
         "w12b": w12b, "w12f": w12f}
        for c in range(NCORES)
    ]


def kernel(x1, x2, W1, b1, W2, W3):
    if not _nc_cache:
        _nc_cache.append(build_nc())
    nc = _nc_cache[0]

    c_b1 = float(np.asarray(b1, dtype=np.float64).sum())
    in_maps = _in_maps(x1, x2, W1, W2, W3)

    res = run_bass_kernel_spmd(nc, in_maps, core_ids=list(range(NCORES)))
    out = np.concatenate(
        [res.results[c]["out"].reshape(-1) for c in range(NCORES)])
    return (out + np.float32(c_b1)).astype(np.float32)
